# revision 1
# baseline (speedup 1.0000x reference)
"""Trainium2 Bass kernel for nn_Classifier0 (quadrant-sum classifier).

reference:
    agg[n, q]  = quadrant sums of x[n] (512x512, quadrants of 256x256)
    w          = g * v[..., 0] / ||v||            [4, 4]
    y          = agg[:, :, None] * w + b_fgl      [N, 4, 4]
    out        = y.reshape(N, 16) @ W_fc.T + b_fc [N, 10]

Algebraic refactor (exact in real arithmetic):
    out[n, c] = sum_q agg[n, q] * A[q, c] + cc[c]
      A[q, c] = sum_j w[q, j] * W_fc[c, 4q + j]         (4 x 10, host, fp64)
      cc[c]   = b_fgl.ravel() @ W_fc[c] + b_fc[c]       (10, host, fp64)

Device work (data-parallel, 32 samples per core):
    - per sample: DMA x[n] as [128, 2048] (partition = 4 consecutive image
      rows), DVE reduces the left 256 columns, ACT (activation Copy with
      accum_out) reduces the right 256 columns -> bufL/bufR [128, 32]
    - partition p < 64 is the top half of the image, so quadrant sums are a
      matmul contraction over partitions:
        out = bufL^T @ AL + bufR^T @ AR + ones^T @ cc   (PSUM [32, 10])
      with AL[p] = A[TL] or A[BL], AR[p] = A[TR] or A[BR] by p < 64.
"""

import numpy as np

N, S = 256, 512
H = S // 2
NCORES = 8
NPC = N // NCORES  # samples per core
NCLS = 10
ROWS_PER_PART = 4  # image rows per SBUF partition
FREE = S * ROWS_PER_PART  # 2048
PARTS = S // ROWS_PER_PART  # 128

_PROGRAM_CACHE = {}


def _build_program():
    from contextlib import ExitStack

    import concourse.bacc as bacc
    import concourse.mybir as mybir
    import concourse.tile as tile

    nc = bacc.Bacc("TRN2", target_bir_lowering=False, debug=False)
    dt = mybir.dt.float32

    x_t = nc.dram_tensor("x", [NPC, PARTS, FREE], dt, kind="ExternalInput")
    wal_t = nc.dram_tensor("wal", [PARTS, NCLS], dt, kind="ExternalInput")
    war_t = nc.dram_tensor("war", [PARTS, NCLS], dt, kind="ExternalInput")
    ccb_t = nc.dram_tensor("ccb", [1, NCLS], dt, kind="ExternalInput")
    y_t = nc.dram_tensor("y", [NPC, NCLS], dt, kind="ExternalOutput")

    with tile.TileContext(nc) as tc, ExitStack() as ctx:
        xpool = ctx.enter_context(tc.tile_pool(name="xp", bufs=8))
        spool = ctx.enter_context(tc.tile_pool(name="sp", bufs=2))
        cpool = ctx.enter_context(tc.tile_pool(name="cp", bufs=1))
        ppool = ctx.enter_context(tc.tile_pool(name="pp", bufs=1, space="PSUM"))

        x_ap = x_t.ap()

        bufL = cpool.tile([PARTS, NPC], dt)
        bufR = cpool.tile([PARTS, NPC], dt)
        wal = cpool.tile([PARTS, NCLS], dt)
        nc.sync.dma_start(wal[:], wal_t.ap())
        war = cpool.tile([PARTS, NCLS], dt)
        nc.sync.dma_start(war[:], war_t.ap())
        ccb = cpool.tile([1, NCLS], dt)
        nc.sync.dma_start(ccb[:], ccb_t.ap())
        ones1 = cpool.tile([1, NPC], dt)
        nc.vector.memset(ones1[:], 1.0)

        for s in range(NPC):
            xt = xpool.tile([PARTS, FREE], dt)
            nc.sync.dma_start(xt[:], x_ap[s])
            xv = xt[:].rearrange("p (r c) -> p r c", r=ROWS_PER_PART)
            # left 256 columns of each of the 4 rows in this partition
            nc.vector.tensor_reduce(
                bufL[:, s : s + 1],
                xv[:, :, 0:H],
                axis=mybir.AxisListType.XY,
                op=mybir.AluOpType.add,
            )
            # right half on the scalar engine via activation accumulate
            scratch = spool.tile([PARTS, ROWS_PER_PART * H], dt)
            sv = scratch[:].rearrange("p (r c) -> p r c", r=ROWS_PER_PART)
            nc.scalar.activation(
                sv,
                xv[:, :, H:S],
                mybir.ActivationFunctionType.Copy,
                accum_out=bufR[:, s : s + 1],
            )

        psum = ppool.tile([NPC, NCLS], dt)
        nc.tensor.matmul(psum[:], lhsT=bufL[:], rhs=wal[:], start=True, stop=False)
        nc.tensor.matmul(psum[:], lhsT=bufR[:], rhs=war[:], start=False, stop=False)
        nc.tensor.matmul(psum[:], lhsT=ones1[:], rhs=ccb[:], start=False, stop=True)

        out_sb = cpool.tile([NPC, NCLS], dt)
        nc.vector.tensor_copy(out_sb[:], psum[:])
        nc.sync.dma_start(y_t.ap(), out_sb[:])

    nc.compile()
    return nc


def _host_params(v, g, b_fgl, W_fc, b_fc):
    """Fold the tiny params into AL/AR [128, 10] and cc [1, 10] (fp64 host)."""
    v64 = v.astype(np.float64)
    w = g.astype(np.float64) * (v64[..., 0] / np.linalg.norm(v64, axis=-1))  # [4,4]
    A = np.einsum("qj,cqj->qc", w, W_fc.astype(np.float64).reshape(NCLS, 4, 4))
    cc = b_fgl.astype(np.float64).reshape(-1) @ W_fc.astype(np.float64).T
    cc = cc + b_fc.astype(np.float64)

    # quadrant ids: 0=TL, 1=BL, 2=BR, 3=TR; partition p < 64 -> top half rows
    AL = np.empty((PARTS, NCLS), np.float64)
    AR = np.empty((PARTS, NCLS), np.float64)
    AL[: PARTS // 2] = A[0]  # top rows, left cols
    AL[PARTS // 2 :] = A[1]  # bottom rows, left cols
    AR[: PARTS // 2] = A[3]  # top rows, right cols
    AR[PARTS // 2 :] = A[2]  # bottom rows, right cols
    return (
        np.ascontiguousarray(AL, dtype=np.float32),
        np.ascontiguousarray(AR, dtype=np.float32),
        np.ascontiguousarray(cc.reshape(1, NCLS), dtype=np.float32),
    )


def _run(inputs, trace=False):
    from concourse.bass_utils import run_bass_kernel_spmd

    if "nc" not in _PROGRAM_CACHE:
        _PROGRAM_CACHE["nc"] = _build_program()
    nc = _PROGRAM_CACHE["nc"]

    x = np.asarray(inputs["x"], dtype=np.float32)
    AL, AR, cc = _host_params(
        np.asarray(inputs["v"], np.float32),
        np.asarray(inputs["g"], np.float32),
        np.asarray(inputs["b_fgl"], np.float32),
        np.asarray(inputs["W_fc"], np.float32),
        np.asarray(inputs["b_fc"], np.float32),
    )

    x_sh = np.ascontiguousarray(x).reshape(NCORES, NPC, PARTS, FREE)
    in_maps = [
        {"x": x_sh[i], "wal": AL, "war": AR, "ccb": cc} for i in range(NCORES)
    ]
    res = run_bass_kernel_spmd(
        nc, in_maps, list(range(NCORES)), trace=trace
    )
    y = np.concatenate([res.results[i]["y"] for i in range(NCORES)], axis=0)
    return y, res.exec_time_ns


def kernel(**inputs) -> np.ndarray:
    y, _ = _run(inputs, trace=False)
    return y
